# revision 39
# baseline (speedup 1.0000x reference)
"""Differential multi-head attention on 8 Trainium2 NeuronCores.

Sharding: core p owns head pair (p, p+8) for both batches (tensor parallel
over the 8 differential head pairs). lambda scalars are folded into the
output-projection weights on the host. Host sums the 8 partial outputs.

Layout per core (hd = 64, pair cols = 128, T = B*N = 4096 tokens):
  xT      [128, 8t, 8c, 512]  x transposed, chunk-contiguous, fp16
  QT, KT  [128, 4096]    projected q/k transposed; rows 0:64 = head p,
                         rows 64:128 = head p+8
  V       [4096, 130]    token-partition layout, cols [h1(64) | 1 | h2(64) | 1]
  S.T     [k, q] chunks  via matmul(lhsT=KT slice, rhs=QT slice), K=64
  P.T     exp(S.T/8)     ACT, written as fp16
  OT_aug  [65, 512]      psum accum over 16 k-chunks: rows 0:64 = (P@V).T,
                         row 64 = softmax denominators
  out.T   [1024, 4096]   = Wcomb.T @ OcombT, partial (fp16); summed on host

Schedule: attention groups (2 slots of S -> exp -> PV) stream continuously;
projections / normalization / output projection are interleaved as filler to
keep the PE dense.  Reciprocal broadcast across partitions is done with a
K=1 matmul (ones outer product) instead of a DRAM round-trip.  Dummy
matmuls at the start hold the PE HAM clock gate open while inputs stream.
"""
import numpy as np

import concourse.bacc as bacc
import concourse.bass as bass
import concourse.tile as tile
import concourse.mybir as mybir
from concourse.bass_utils import run_bass_kernel_spmd

F32 = mybir.dt.float32
F32R = mybir.dt.float32r
F16 = mybir.dt.float16

EMBED = 1024
H2 = 8
HD = 64
B = 2
N = 2048
T = B * N  # 4096
NCORES = 8
LAMBDA_INIT = 0.8
SCALE = HD ** -0.5

WARMUP_MM = 44

TRACE = False
LAST_RESULT = [None]

_compiled = [None]


def ts(i, size):
    return slice(i * size, (i + 1) * size)


def _build():
    nc = bacc.Bacc("TRN2", target_bir_lowering=False, debug=False, num_devices=NCORES)

    xT_d = nc.dram_tensor("xT", [128, 8, 8, 512], F16, kind="ExternalInput").ap()
    wq_d = nc.dram_tensor("wq", [128, 8, 128], F16, kind="ExternalInput").ap()
    wk_d = nc.dram_tensor("wk", [128, 8, 128], F16, kind="ExternalInput").ap()
    wv_d = nc.dram_tensor("wv", [128, 8, 128], F16, kind="ExternalInput").ap()
    wc_d = nc.dram_tensor("wcomb", [128, 1024], F16, kind="ExternalInput").ap()
    bq_d = nc.dram_tensor("bq", [128, 1], F32, kind="ExternalInput").ap()
    bk_d = nc.dram_tensor("bk", [128, 1], F32, kind="ExternalInput").ap()
    bva_d = nc.dram_tensor("bvaug", [128, 130], F32, kind="ExternalInput").ap()
    outT_d = nc.dram_tensor("outT", [EMBED, T], F16, kind="ExternalOutput").ap()

    with tile.TileContext(nc) as tc:
        with (
            tc.tile_pool(name="consts", bufs=1) as consts,
            tc.tile_pool(name="xp", bufs=8) as xp,
            tc.tile_pool(name="qkv", bufs=1) as qkv,
            tc.tile_pool(name="ptp", bufs=2) as ptp,
            tc.tile_pool(name="stage", bufs=3) as stage,
            tc.tile_pool(name="bcp", bufs=4) as bcp,
            tc.tile_pool(name="outp", bufs=8) as outp,
            tc.tile_pool(name="ps_st", bufs=2, space="PSUM") as ps_st,
            tc.tile_pool(name="ps_ot", bufs=1, space="PSUM") as ps_ot,
            tc.tile_pool(name="ps_c", bufs=2, space="PSUM") as ps_c,
        ):
            # ---- constant tiles ----
            wq_t = consts.tile([128, 8, 128], F16, name="wq_t")
            wk_t = consts.tile([128, 8, 128], F16, name="wk_t")
            wv_t = consts.tile([128, 8, 128], F16, name="wv_t")
            wc_t = consts.tile([128, 1024], F16, name="wc_t")
            bq_t = consts.tile([128, 1], F32, name="bq_t")
            bk_t = consts.tile([128, 1], F32, name="bk_t")
            bva_t = consts.tile([128, 130], F32, name="bva_t")
            warm_t = consts.tile([128, 128], F16, name="warm_t")
            ones_t = consts.tile([33, 64], F16, name="ones_t")

            qt_t = qkv.tile([128, T], F16, name="qt_t")
            kt_t = qkv.tile([128, T], F16, name="kt_t")
            v_t = qkv.tile([128, 32, 200], F16, name="v_t")
            ot_t = qkv.tile([128, B, N], F16, name="ot_t")
            oc_t = qkv.tile([128, B, N], F16, name="oc_t")

            # ---- HAM warm-up: dummy matmuls while input DMAs stream ----
            nc.gpsimd.memset(warm_t, 0.0)
            nc.gpsimd.memset(ones_t, 1.0)
            psw = ps_c.tile([128, 512], F32, name="ps_c")
            for i in range(WARMUP_MM):
                nc.tensor.matmul(
                    psw[:, 0:128], warm_t, warm_t,
                    start=(i == 0), stop=(i == WARMUP_MM - 1),
                )

            # ---- input DMAs: earliest-needed first, spread across queues ----
            nc.sync.dma_start(out=wq_t, in_=wq_d)
            xt_tiles = {}

            def xt_fetch(t, eng):
                xt = xp.tile([128, 8, 512], F16, name="xt")
                eng.dma_start(out=xt, in_=xT_d[:, t, :, :])
                xt_tiles[t] = xt

            # chunk 0 in two halves so the first projection matmuls can
            # start as soon as the first half lands (subtile deps)
            xt0 = xp.tile([128, 8, 512], F16, name="xt")
            nc.sync.dma_start(out=xt0[:, 0:4, :], in_=xT_d[:, 0, 0:4, :])
            nc.sync.dma_start(out=xt0[:, 4:8, :], in_=xT_d[:, 0, 4:8, :])
            xt_tiles[0] = xt0
            nc.scalar.dma_start(out=wk_t, in_=wk_d)
            nc.scalar.dma_start(out=bva_t, in_=bva_d)
            nc.scalar.dma_start(out=bq_t, in_=bq_d)
            nc.scalar.dma_start(out=bk_t, in_=bk_d)
            xt_fetch(1, nc.sync)
            nc.scalar.dma_start(out=wv_t, in_=wv_d)
            xt_fetch(2, nc.sync)
            xt_fetch(3, nc.sync)
            nc.scalar.dma_start(out=wc_t, in_=wc_d)
            for t in range(4, 8):
                xt_fetch(t, nc.sync)
            nc.vector.memset(v_t[:, :, 130:200], 0.0)

            # ---- primitive work units ----
            def proj_one(t, wt, dst, bias):
                """QK projection of token chunk t (512 tokens) -> dst slice."""
                xt = xt_tiles[t]
                psq = ps_c.tile([128, 512], F32, name="ps_c")
                for f in range(8):
                    nc.tensor.matmul(
                        psq, wt[:, f, :], xt[:, f, :],
                        start=(f == 0), stop=(f == 7),
                    )
                nc.vector.tensor_scalar_add(dst[:, ts(t, 512)], psq, bias)

            def proj_q(t):
                proj_one(t, wq_t, qt_t, bq_t)

            def proj_k(t):
                proj_one(t, wk_t, kt_t, bk_t)

            def proj_v_half(t, half):
                """Project half (2 of 4 sub-chunks) of token chunk t -> V."""
                xt = xt_tiles[t]
                for sub in (2 * half, 2 * half + 1):
                    c = t * 4 + sub
                    psv = ps_c.tile([128, 512], F32, name="ps_c")
                    for f in range(8):
                        nc.tensor.matmul(
                            psv[:, 0:128], xt[:, f, ts(sub, 128)], wv_t[:, f, :],
                            start=(f == 0), stop=(f == 7),
                        )
                    nc.vector.tensor_add(v_t[:, c, 0:64], psv[:, 0:64], bva_t[:, 0:64])
                    nc.vector.tensor_add(v_t[:, c, 65:129], psv[:, 64:128], bva_t[:, 65:129])
                v0 = t * 4 + 2 * half
                nc.vector.tensor_copy(
                    v_t[:, v0:v0 + 2, 64:65],
                    bva_t[:, None, 64:65].broadcast_to([128, 2, 1]),
                )
                nc.vector.tensor_copy(
                    v_t[:, v0:v0 + 2, 129:130],
                    bva_t[:, None, 129:130].broadcast_to([128, 2, 1]),
                )

            def proj_v(t):
                proj_v_half(t, 0)
                proj_v_half(t, 1)

            d33_tiles = {}

            def drain_accums(b, qc, otps, last=False):
                """PSUM accumulators -> ot_t (SBUF, fp32) via staging; the
                two denominator rows (row 64) go into a [33,512] tile at
                rows 0/32 so both heads get legal matmul tile positions."""
                if last:
                    # shortest tail: stage both halves in parallel on
                    # vector/scalar, pull denominators straight off PSUM
                    # with partition-shifted copies, skip the ot_t round
                    # trip (norm2 reads the staging tiles directly)
                    d33 = bcp.tile([33, 512], F32, name="d33f", tag="d33f",
                                   bufs=2)
                    stg0 = stage.tile([64, 512], F16, name="stg")
                    stg1 = stage.tile([64, 512], F16, name="stg")
                    nc.vector.tensor_copy(stg0, otps[0][0:64, :])
                    nc.scalar.copy(stg1, otps[1][0:64, :])
                    nc.vector.tensor_copy(d33[32:33, :], otps[1][64:65, :])
                    nc.scalar.copy(d33[0:1, :], otps[0][64:65, :])
                    d33_tiles[(b, qc)] = (d33, False)
                    last_stgs[0] = (stg0, stg1)
                    return
                d33 = bcp.tile([33, 512], F16, name="d33", tag="d33", bufs=2)
                stgs = []
                for h in (0, 1):
                    stg = stage.tile([65, 512], F16, name="stg")
                    nc.vector.tensor_copy(stg, otps[h][0:65, :])
                    stgs.append(stg)
                for h in (0, 1):
                    nc.gpsimd.dma_start(
                        out=d33[32 * h:32 * h + 1, :], in_=stgs[h][64:65, :])
                for h in (0, 1):
                    nc.gpsimd.dma_start(
                        out=ot_t[h * 64:(h + 1) * 64, b, ts(qc, 512)],
                        in_=stgs[h][0:64, :],
                    )
                d33_tiles[(b, qc)] = (d33, True)

            rd_tiles = {}
            last_stgs = [None]

            def norm1(b, qc):
                """reciprocal of both denominator rows in one shot."""
                d33, needs_f32 = d33_tiles.pop((b, qc))
                if needs_f32:
                    d33f = bcp.tile([33, 512], F32, name="d33f", tag="d33f",
                                    bufs=2)
                    nc.vector.tensor_copy(d33f, d33)
                    d33 = d33f
                rd33 = bcp.tile([33, 512], F32, name="rd33", tag="rd33", bufs=2)
                rd16 = bcp.tile([33, 512], F16, name="rd16", tag="rd16", bufs=2)
                nc.vector.reciprocal_approx_fast(rd33, d33)
                nc.vector.tensor_copy(rd16, rd33)
                rd_tiles[(b, qc)] = rd16

            def norm2(b, qc):
                """broadcast 1/denominator across partitions with a K=1
                matmul, then oc = ot * bc."""
                rd16 = rd_tiles.pop((b, qc))
                psbc = ps_c.tile([128, 512], F32, name="ps_c")
                for h in (0, 1):
                    nc.tensor.matmul(
                        psbc[h * 64:(h + 1) * 64, :],
                        ones_t[32 * h:32 * h + 1, :],
                        rd16[32 * h:32 * h + 1, :],
                        start=True, stop=True,
                    )
                if last_stgs[0] is not None and (b, qc) == (1, 3):
                    for h in (0, 1):
                        nc.vector.tensor_mul(
                            oc_t[h * 64:(h + 1) * 64, b, ts(qc, 512)],
                            last_stgs[0][h],
                            psbc[h * 64:(h + 1) * 64, :],
                        )
                else:
                    nc.vector.tensor_mul(
                        oc_t[:, b, ts(qc, 512)], ot_t[:, b, ts(qc, 512)], psbc
                    )

            def outproj_m(b, qc, m, on_scalar=False, ps_pool=None, ps_name="ps_c",
                          dma_eng=None):
                pso = (ps_pool or ps_c).tile([128, 512], F32, name=ps_name)
                nc.tensor.matmul(
                    pso, wc_t[:, ts(m, 128)], oc_t[:, b, ts(qc, 512)],
                    start=True, stop=True,
                )
                so = outp.tile([128, 512], F16, name="so")
                if on_scalar:
                    nc.scalar.copy(so, pso)
                else:
                    nc.vector.tensor_copy(so, pso)
                (dma_eng or nc.sync).dma_start(
                    out=outT_d[ts(m, 128), b * N + qc * 512: b * N + (qc + 1) * 512],
                    in_=so,
                )

            # ---- filler queue ----
            post = []

            def push_chunk_post(b, qc):
                post.append(lambda: norm1(b, qc))
                post.append(lambda: norm2(b, qc))
                for m in range(8):
                    post.append(lambda m=m: outproj_m(b, qc, m))

            # forced just-in-time projection work per (b, qc) phase, keyed by
            # group index g0.  Demand: Q(c) before phase (b, qc) with
            # b*4+qc == c starts; K(c)/V(c) before batch c//4's scores/PV
            # reach k-chunk 4*(c%4).
            jit = {
                (0, 0): {4: [lambda: proj_k(1)],
                         6: [lambda: proj_v_half(1, 0)],
                         10: [lambda: proj_v_half(1, 1)],
                         12: [lambda: proj_k(2)],
                         16: [lambda: proj_v_half(2, 0)],
                         18: [lambda: proj_v_half(2, 1)],
                         20: [lambda: proj_k(3)],
                         24: [lambda: proj_v_half(3, 0)],
                         26: [lambda: proj_v_half(3, 1)]},
                (0, 1): {4: [lambda: proj_q(2)],
                         14: [lambda: proj_k(4)]},
                (0, 2): {4: [lambda: proj_v_half(4, 0)],
                         14: [lambda: proj_q(3)],
                         24: [lambda: proj_v_half(4, 1)]},
                (0, 3): {4: [lambda: proj_k(5)],
                         12: [lambda: proj_v_half(5, 0)],
                         20: [lambda: proj_q(4)],
                         26: [lambda: proj_v_half(5, 1)]},
                (1, 0): {2: [lambda: proj_k(6)],
                         6: [lambda: proj_v_half(6, 0)],
                         10: [lambda: proj_v_half(6, 1)],
                         14: [lambda: proj_k(7)],
                         18: [lambda: proj_v_half(7, 0)],
                         22: [lambda: proj_v_half(7, 1)],
                         26: [lambda: proj_q(5)]},
                (1, 1): {4: [lambda: proj_q(6)]},
                (1, 2): {4: [lambda: proj_q(7)]},
                (1, 3): {},
            }
            # filler pop rate in units per 2-slot group, per phase
            pop_rate = {
                (0, 0): 0.0, (0, 1): 0.5, (0, 2): 0.35, (0, 3): 0.35,
                (1, 0): 0.35, (1, 1): 1.3, (1, 2): 1.3, (1, 3): 1.6,
            }

            # prologue: project QK of chunk 0, V of chunk 0, Q of chunk 1
            proj_q(0)
            proj_k(0)
            proj_v(0)
            proj_q(1)

            for b in range(2):
                for qc in range(4):
                    otps = [
                        ps_ot.tile([128, 512], F32, name=f"ps_ot{h}") for h in (0, 1)
                    ]
                    qoff = b * N + qc * 512
                    slots = [(kc, h) for kc in range(16) for h in (0, 1)]
                    pending = None  # PV work delayed one group (PE FIFO overlap)
                    pops = 0.0
                    for g0 in range(0, 34, 2):
                        if g0 < 32:
                            grp = slots[g0:g0 + 2]
                            st = ps_st.tile([128, 1024], F32, name="ps_st")
                            pt = ptp.tile([128, 1024], F16, name="pt")
                            for i, (kc, h) in enumerate(grp):
                                lo = h * 64
                                koff = b * N + kc * 128
                                nc.tensor.matmul(
                                    st[:, ts(i, 512)],
                                    kt_t[lo:lo + 64, koff:koff + 128],
                                    qt_t[lo:lo + 64, qoff:qoff + 512],
                                    start=True, stop=True,
                                )
                            nc.scalar.activation(
                                pt[:, 0:len(grp) * 512], st[:, 0:len(grp) * 512],
                                mybir.ActivationFunctionType.Exp, scale=SCALE,
                            )
                        if pending is not None:
                            pgrp, ppt = pending
                            for i, (kc, h) in enumerate(pgrp):
                                nc.tensor.matmul(
                                    otps[h],
                                    v_t[:, b * 16 + kc, h * 65:h * 65 + 128],
                                    ppt[:, ts(i, 512)],
                                    start=(kc == 0), stop=(kc == 15),
                                )
                        pending = (grp, pt) if g0 < 32 else None
                        for fn in jit[(b, qc)].get(g0, []):
                            fn()
                        if g0 >= 4:
                            pops += pop_rate[(b, qc)]
                            while post and pops >= 1.0:
                                post.pop(0)()
                                pops -= 1.0
                    drain_accums(b, qc, otps, last=(b == 1 and qc == 3))
                    push_chunk_post(b, qc)

            # epilogue: the last chunk's chain is the critical path; its
            # PSUM->fp16 casts alternate between the idle ACT engine and DVE
            backlog = post[:-10]
            post[-10]()  # norm1(1,3)
            for f in backlog[:2]:
                f()
            post[-9]()   # norm2(1,3)
            for m in range(8):
                # rotate over 4 PSUM slots (attention accumulators are free
                # by now) and both cast engines to keep the chain dense
                if m % 4 < 2:
                    pool, name = ps_ot, f"ps_ot{m % 4}"
                else:
                    pool, name = ps_c, "ps_c"
                outproj_m(1, 3, m, on_scalar=(m % 2 == 0), ps_pool=pool,
                          ps_name=name,
                          dma_eng=(nc.gpsimd if m % 2 else nc.sync))
                for g in backlog[2 + 2 * m: 2 + 2 * (m + 1)]:
                    g()
            for f in backlog[18:]:
                f()

    nc.compile()
    return nc


def kernel(x, Wq, bq, Wk, bk, Wv, bv, Wp, bp,
           lambda_q1, lambda_k1, lambda_q2, lambda_k2):
    x = np.asarray(x, dtype=np.float32)
    Wq, Wk, Wv, Wp = [np.asarray(w, dtype=np.float32) for w in (Wq, Wk, Wv, Wp)]
    bq, bk, bv, bp = [np.asarray(v, dtype=np.float32) for v in (bq, bk, bv, bp)]

    l1 = np.exp(np.minimum(
        (np.asarray(lambda_q1, np.float32) * np.asarray(lambda_k1, np.float32))
        .sum((-1, -2)), 5.0))
    l2 = np.exp(np.minimum(
        (np.asarray(lambda_q2, np.float32) * np.asarray(lambda_k2, np.float32))
        .sum((-1, -2)), 5.0))
    lv = np.float32((l1 - l2 + np.float32(LAMBDA_INIT)).mean())

    # xT chunk-contiguous layout: [p, t, c, n] with 8KB contiguous per
    # (partition, chunk) so each chunk DMA is one descriptor per row
    xT = x.reshape(T, EMBED).T.astype(np.float16)          # [1024, 4096]
    xT4 = np.ascontiguousarray(
        xT.reshape(8, 128, 8, 512).transpose(1, 2, 0, 3))  # [128, 8t, 8c, 512]

    if _compiled[0] is None:
        _compiled[0] = _build()
    nc = _compiled[0]

    in_maps = []
    for p in range(NCORES):
        r1 = slice(p * HD, (p + 1) * HD)          # head p rows/cols
        r2 = slice((8 + p) * HD, (9 + p) * HD)    # head p+8 rows/cols
        wq_p = np.concatenate([Wq[r1], Wq[r2]], 0).T      # [1024, 128]
        wk_p = np.concatenate([Wk[r1], Wk[r2]], 0).T
        wv_p = np.concatenate([Wv[r1], Wv[r2]], 0).T
        wpt1 = Wp[:, r1].T                                 # [64, 1024]
        wpt2 = Wp[:, r2].T
        wcomb = np.concatenate([wpt1, wpt2 - lv * wpt1], 0)  # [128, 1024]
        bva = np.ascontiguousarray(np.broadcast_to(np.concatenate(
            [bv[r1], [1.0], bv[r2], [1.0]]).astype(np.float32)[None, :],
            (128, 130)))

        def wlay(w):  # [1024, 128] -> [128p, 8c, 128m] contiguous
            return np.ascontiguousarray(
                w.reshape(8, 128, 128).transpose(1, 0, 2).astype(np.float16))

        in_maps.append({
            "xT": xT4,
            "wq": wlay(wq_p),
            "wk": wlay(wk_p),
            "wv": wlay(wv_p),
            "wcomb": np.ascontiguousarray(wcomb.astype(np.float16)),
            "bq": np.concatenate([bq[r1], bq[r2]])[:, None].copy(),
            "bk": np.concatenate([bk[r1], bk[r2]])[:, None].copy(),
            "bvaug": np.ascontiguousarray(bva),
        })

    res = run_bass_kernel_spmd(
        nc, in_maps, core_ids=list(range(NCORES)), trace=TRACE,
    )
    LAST_RESULT[0] = res

    outT = res.results[0]["outT"].astype(np.float64)
    for c in range(1, NCORES):
        outT += res.results[c]["outT"]
    out = outT.T.reshape(B, N, EMBED).astype(np.float32) + bp[None, None, :]
    return out
